# revision 3
# baseline (speedup 1.0000x reference)
"""BAD-descriptor kernel for Trainium2 (8 NeuronCores, SPMD over pairs).

The v1 baseline (147-158us) was DMA packet-rate bound (~35.7K packets @
~60ns/pkt/engine): every pair re-gathered two 224x224 windows from a DRAM
box-mean scratch in 896B packets.  This version (~64us) keeps the box-mean
images in SBUF (bf16) in an interleaved-plane layout

  bmp[k, d-1, plane, b, c],  plane E[k]=BMP row 2k, O[k]=2k+1, Es[k]=2k+2

so a window with row-shift sy = 2q+c is partitions q..q+111, planes c..c+1.
Compute-engine APs must start at partition 0/32/64/96 (TRN2 quadrant rule),
so for q>0 the shift is materialized by a partition-shift SBUF->SBUF DMA
"slab" (112 packets x 3KB), deduplicated per (d, q) cell across the core's
64 window terms.  Pairs are ASSIGNED to cores (greedy + swap rebalance) to
minimize the worst core's slab time, since cells shared within a core are
copied once.  q==0 windows read bmp directly; column shift and plane parity
are free-dim offsets.  Per pair either

  DVE:    out = (W1 + (-thr)) - W2      (one scalar_tensor_tensor), or
  PE+ACT: psum = I*W1 + (-I)*W2; ACT drains with bias=-thr (Identity)

with ~7 pairs offloaded to the otherwise-idle PE/ACT to shorten the DVE
stream.  All shifts/radii are computed on the HOST (the offsets are kernel
inputs) and each core's 32-pair schedule is baked into an 8-way tc.Switch
on the partition id -> one SPMD program, no per-pair gathers/values_loads.
Outputs are written as bf16 (halves the flush) and upcast on the host;
end-to-end rel L2 error ~4e-3 vs the 2e-2 gate.

Box-mean build (stage B), pipelined per radius so d=1 pairs start earliest:
bf16 horizontal 7-tap chains on DVE (edge pads on GpSimd), vertical taps +
row-replicate clipping + 1/area baked into bf16 band matrices on PE
(2 K-tiles x 9 (plane,d) matmuls), psum drained to bmp by ACT.  Slab DMAs
dispatch on SP, early output DMAs on ACT, late ones on SP.
"""

import sys

sys.path.insert(0, "/opt/trn_rl_repo")

from contextlib import ExitStack

import numpy as np
import ml_dtypes

import concourse.bass as bass
import concourse.bacc as bacc
import concourse.mybir as mybir
import concourse.tile as tile
from concourse.bass_utils import run_bass_kernel_spmd

B = 2
H = W = 224
P_TOTAL = 256
N_CORES = 8
P_CORE = P_TOTAL // N_CORES  # 32
PAD = 16
RMAX = 3
HP = H + 2 * PAD  # 256 padded rows/cols
XPAD = RMAX  # 3: replicate pad for the +-d box samples
XW = W + 2 * XPAD  # 230
F32 = mybir.dt.float32
BF16 = mybir.dt.bfloat16
NPART = 112
SLAB_SLOTS = 24  # LRU capacity for multi-use (d,q) slabs: 24 x 3KB/partition


def _host_schedule(offset_y1, offset_x1, offset_y2, offset_x2, radii):
    """Per-core pair schedules (d, sy1, sx1, sy2, sx2).

    Pairs are ASSIGNED to cores greedily so that pairs sharing a shifted
    window cell (d, sy) land on the same core — each distinct cell costs one
    229KB slab copy, the dominant stage-C DMA traffic.  q==0 cells are free
    (direct bmp reads).
    """

    def prep(off):
        fo = np.floor(np.asarray(off, np.float32).astype(np.float64))
        return (np.clip(fo, -PAD, PAD) + PAD).astype(np.int64)  # [0,32]

    sy1, sx1 = prep(offset_y1), prep(offset_x1)
    sy2, sx2 = prep(offset_y2), prep(offset_x2)
    d = np.clip(np.asarray(radii).astype(np.int64), 1, RMAX)

    def cells(p):
        s = set()
        for sy in (int(sy1[p]), int(sy2[p])):
            if (sy >> 1) != 0:
                s.add((int(d[p]), sy >> 1))
        return s

    from collections import Counter
    freq = Counter()
    for p in range(P_TOTAL):
        for cell in cells(p):
            freq[cell] += 1

    def slab_bytes(take):
        """Time proxy (ns-ish): dispatch serialization + transfer bytes."""
        cnt = Counter()
        for p in take:
            for cell in cells(p):
                cnt[cell] += 1
        b = sum(344064 if v > 1 else 200704 for v in cnt.values())
        return len(cnt) * 900 + b // 300

    # sequential greedy builds tight clusters (later cores get scattered
    # leftovers), then pairwise swaps rebalance the worst core (exec time
    # is the MAX over cores)
    remaining = set(range(P_TOTAL))
    takes = []
    for c in range(N_CORES):
        take, open_cells = [], set()
        while len(take) < P_CORE:
            best, best_key = None, None
            for p in remaining:
                cs = cells(p)
                reuse = sum(1 for x in cs if x in open_cells)
                new = [x for x in cs if x not in open_cells]
                key = (reuse, -len(new), sum(freq[x] for x in new))
                if best_key is None or key > best_key:
                    best, best_key = p, key
            take.append(best)
            remaining.discard(best)
            for x in cells(best):
                open_cells.add(x)
                freq[x] -= 1
        takes.append(take)

    cur = [slab_bytes(t) for t in takes]
    for _ in range(300):
        hi = int(np.argmax(cur))
        best = None
        for lo in range(N_CORES):
            if lo == hi:
                continue
            for i in range(P_CORE):
                for jj in range(P_CORE):
                    t1 = takes[hi][:i] + takes[hi][i + 1:] + [takes[lo][jj]]
                    t2 = takes[lo][:jj] + takes[lo][jj + 1:] + [takes[hi][i]]
                    m = max(slab_bytes(t1), slab_bytes(t2))
                    if m < max(cur[hi], cur[lo]) and (
                            best is None or m < best[0]):
                        best = (m, lo, i, jj)
        if best is None:
            break
        _, lo, i, jj = best
        takes[hi][i], takes[lo][jj] = takes[lo][jj], takes[hi][i]
        cur[hi] = slab_bytes(takes[hi])
        cur[lo] = slab_bytes(takes[lo])

    cores = []
    for c in range(N_CORES):
        # within a core, order pairs by radius so stage C can start as soon
        # as the d=1 planes are built, preserving cluster adjacency second
        order = np.array(sorted(takes[c], key=lambda p: int(d[p])))
        sched = tuple(
            (int(d[p]), int(sy1[p]), int(sx1[p]), int(sy2[p]), int(sx2[p]))
            for p in order
        )
        cores.append((order, sched))
    return cores


def _band_matrices() -> np.ndarray:
    """sdt[kt, kr, 3*pi + d-1, m]: vertical (2d+1)-tap band matrices with the
    row-replicate clipping and the 1/(2d+1)^2 box area baked in.

    Plane pi in {0:E, 1:O, 2:Es} holds BMP row r = 2m + pi at partition m;
    BMP row r represents BM row h = clip(r-16, 0, 223) whose box mean is
    (1/area) * sum_i hs_d[clip(h+i, 0, 223)].  K-tile 0 = x rows 0..127,
    K-tile 1 = x rows 96..223 (coefficients split at row 128).
    """
    sdt = np.zeros((2, 128, 9, 128), np.float32)
    for dd in (1, 2, 3):
        inv_area = 1.0 / float((2 * dd + 1) ** 2)
        for pi in range(3):
            col = 3 * pi + (dd - 1)
            for m in range(128):
                r = 2 * m + pi
                h = min(max(r - PAD, 0), H - 1)
                for i in range(-dd, dd + 1):
                    xr = min(max(h + i, 0), H - 1)
                    if xr < 128:
                        sdt[0, xr, col, m] += inv_area
                    else:
                        sdt[1, xr - 96, col, m] += inv_area
    return sdt.astype(ml_dtypes.bfloat16)


def build_device_program(nc: bacc.Bacc, schedules):
    x_ap = nc.dram_tensor("x", [B, H, W], BF16, kind="ExternalInput").ap()
    sdt_ap = nc.dram_tensor("sdt", [2, 128, 9, 128], BF16,
                            kind="ExternalInput").ap()
    eye_ap = nc.dram_tensor("eye", [NPART, 2, NPART], BF16,
                            kind="ExternalInput").ap()
    thr_ap = nc.dram_tensor("thr", [1, P_CORE], BF16,
                            kind="ExternalInput").ap()
    out_ap = nc.dram_tensor("out", [P_CORE, H, B, W], BF16,
                            kind="ExternalOutput").ap()
    with tile.TileContext(nc) as tc:
        build_kernel(tc, out_ap, x_ap, sdt_ap, eye_ap, thr_ap, schedules)
    return nc


def build_kernel(tc, out_ap, x_ap, sdt_ap, eye_ap, thr_ap, schedules):
    nc = tc.nc
    EngT = mybir.EngineType
    Alu = mybir.AluOpType
    Act = mybir.ActivationFunctionType

    ctx = ExitStack()
    const_pool = ctx.enter_context(tc.tile_pool(name="const", bufs=1))
    work_pool = ctx.enter_context(tc.tile_pool(name="work", bufs=1))
    psum_pool = ctx.enter_context(tc.tile_pool(name="psum", bufs=2,
                                               space="PSUM"))
    slab_pool = ctx.enter_context(tc.tile_pool(name="slab", bufs=1))
    o_pool = ctx.enter_context(tc.tile_pool(name="outt", bufs=5))

    # ---------------- inputs / constants ----------------
    thr_bc = const_pool.tile([NPART, P_CORE], BF16, tag="thr_bc")
    nc.scalar.dma_start(out=thr_bc[:],
                        in_=thr_ap[0:1, :].to_broadcast((NPART, P_CORE)))

    part_rows = ((0, 128), (96, 128))  # x K-tiles (overlapping rows 96..127)
    xts = []
    for j, (r0, nr) in enumerate(part_rows):
        xt = work_pool.tile([nr, B, XW], BF16, tag=f"xt{j}")
        eng = nc.sync if j == 0 else nc.scalar
        eng.dma_start(out=xt[:, :, XPAD:XPAD + W],
                      in_=x_ap[:, r0:r0 + nr, :].rearrange("b r w -> r b w"))
        nc.vector.tensor_copy(
            out=xt[:, :, 0:XPAD],
            in_=xt[:, :, XPAD:XPAD + 1].to_broadcast((nr, B, XPAD)))
        nc.vector.tensor_copy(
            out=xt[:, :, XPAD + W:],
            in_=xt[:, :, XPAD + W - 1:XPAD + W].to_broadcast((nr, B, XPAD)))
        xts.append(xt)

    sdt_sb = const_pool.tile([128, 2, 9, 128], BF16, tag="sdt")
    nc.sync.dma_start(out=sdt_sb[:, 0], in_=sdt_ap[0])
    nc.scalar.dma_start(out=sdt_sb[:, 1], in_=sdt_ap[1])
    eye_sb = const_pool.tile([NPART, 2, NPART], BF16, tag="eye")
    nc.sync.dma_start(out=eye_sb[:], in_=eye_ap[:])

    # ------- box-mean build, pipelined per radius so d=1 lands first ------
    # hs is computed at VALID centers c in [0,224) (samples clip via the
    # replicate-padded xt); the 16-wide window margins replicate the EDGE
    # hs value (reference clips the box center, then samples around it).
    # bmp[k, d-1, plane, b, c]: plane-adjacent-within-d so a (d, parity) slab
    # source bmp[q:q+112, d-1, c:c+2, :, :] is 2KB contiguous per partition.
    bmp = const_pool.tile([128, 3, 3, B, HP], BF16, tag="bmp")
    hprev = [None, None]
    CHAIN = {1: (2, 4), 2: (1, 5), 3: (0, 6)}  # xt slice offsets per radius
    for dd in (1, 2, 3):
        hsb = []
        for j, (r0, nr) in enumerate(part_rows):
            xt = xts[j]
            sl = lambda o: xt[:, :, o:o + W]
            oa, ob = CHAIN[dd]
            hf = work_pool.tile([nr, B, HP], BF16, tag=f"h{dd}_{j}")
            tt = work_pool.tile([nr, B, W], BF16, tag=f"t{dd}_{j}")
            hv = lambda h: h[:, :, PAD:PAD + W]
            nc.vector.tensor_tensor(out=tt[:], in0=sl(oa), in1=sl(ob),
                                    op=Alu.add)
            if dd == 1:
                nc.vector.tensor_tensor(out=hv(hf), in0=tt[:], in1=sl(3),
                                        op=Alu.add)
            else:
                nc.vector.tensor_tensor(out=hv(hf), in0=hv(hprev[j]),
                                        in1=tt[:], op=Alu.add)
            hprev[j] = hf
            nc.gpsimd.tensor_copy(
                out=hf[:, :, 0:PAD],
                in_=hf[:, :, PAD:PAD + 1].to_broadcast((nr, B, PAD)))
            nc.gpsimd.tensor_copy(
                out=hf[:, :, PAD + W:],
                in_=hf[:, :, PAD + W - 1:PAD + W].to_broadcast((nr, B, PAD)))
            hsb.append(hf)
        for pi in range(3):
            col = 3 * pi + (dd - 1)
            ps = psum_pool.tile([128, B * HP], F32, tag=f"ps{pi % 2}")
            nc.tensor.matmul(out=ps[:], lhsT=sdt_sb[:, 0, col, :],
                             rhs=hsb[0][:].rearrange("r b c -> r (b c)"),
                             start=True, stop=False)
            nc.tensor.matmul(out=ps[:], lhsT=sdt_sb[:, 1, col, :],
                             rhs=hsb[1][:].rearrange("r b c -> r (b c)"),
                             start=False, stop=True)
            nc.scalar.activation(bmp[:, dd - 1, pi, :, :],
                                 ps[:].rearrange("m (b c) -> m b c", b=B),
                                 Act.Copy, scale=1.0)

    # ---------------- stage C: per-core static schedules via Switch -------
    pid = nc.partition_id(engines=[EngT.DVE, EngT.SP, EngT.Activation, EngT.PE])
    GROUP = 4  # pairs per merged output DMA
    for case in tc.Switch(pid, n=N_CORES):
        sched = schedules[case]
        # slab usage counts: single-use slabs are parity/col-sliced,
        # multi-use slabs hold all 3 planes of one radius at one q (3KB/
        # partition, one dispatch serves both parities and any sx)
        from collections import Counter
        cnt = Counter()
        for (dd, sy1, sx1, sy2, sx2) in sched:
            for sy in (sy1, sy2):
                if (sy >> 1) != 0:
                    cnt[(dd, sy >> 1)] += 1
        cache = {}
        lru = []  # keys, most recent last
        free_slots = list(range(SLAB_SLOTS))

        def window(dd, sy, sx):
            q, c = sy >> 1, sy & 1
            if q == 0:
                return bmp[0:NPART, dd - 1, c:c + 2, :, sx:sx + W]
            key = (dd, q)
            if cnt[key] == 1:
                st = slab_pool.tile([NPART, 2, B, W], BF16, tag="ss",
                                    bufs=6, name="sslab")
                nc.sync.dma_start(
                    out=st[:],
                    in_=bmp[q:q + NPART, dd - 1, c:c + 2, :, sx:sx + W])
                return st[:]
            if key in cache:
                lru.remove(key)
                lru.append(key)
                slab = cache[key]
            else:
                if free_slots:
                    slot = free_slots.pop()
                else:
                    old = lru.pop(0)
                    slot = cache.pop(old)[0]
                st = slab_pool.tile([NPART, 3, B, HP], BF16,
                                    tag=f"sl{slot}", bufs=1, name=f"slab{slot}")
                nc.sync.dma_start(out=st[:],
                                  in_=bmp[q:q + NPART, dd - 1, :, :, :])
                cache[key] = (slot, st)
                lru.append(key)
                slab = cache[key]
            return slab[1][:, c:c + 2, :, sx:sx + W]

        # groups of 4 early; smaller at the tail so the final out-DMA (which
        # serializes after the last STT) is short
        bounds = list(range(0, 28, 2)) + [28, 29, 30, 31]
        for bi, g0 in enumerate(bounds):
            g1 = bounds[bi + 1] if bi + 1 < len(bounds) else len(sched)
            grp = sched[g0:g1]
            og = o_pool.tile([NPART, len(grp), 2, B, W], BF16, tag="o")
            for gi, (dd, sy1, sx1, sy2, sx2) in enumerate(grp):
                j = g0 + gi
                in0 = window(dd, sy1, sx1)
                in1 = window(dd, sy2, sx2)
                if j % 5 == 2 or j in (29, 31):
                    # offload to PE: psum = I*W1 + (-I)*W2 (one bank per
                    # plane-half), ACT drains with bias = -thr + bf16 cast
                    for t in range(2):
                        pp = psum_pool.tile([NPART, B, W], F32,
                                            tag=f"pp{t}", bufs=2)
                        nc.tensor.matmul(out=pp[:], lhsT=eye_sb[:, 0, :],
                                         rhs=in0[:, t], start=True, stop=False)
                        nc.tensor.matmul(out=pp[:], lhsT=eye_sb[:, 1, :],
                                         rhs=in1[:, t], start=False, stop=True)
                        nc.scalar.activation(og[:, gi, t], pp[:], Act.Identity,
                                             bias=thr_bc[0:NPART, j:j + 1],
                                             scale=1.0)
                else:
                    nc.vector.scalar_tensor_tensor(
                        out=og[:, gi], in0=in0,
                        scalar=thr_bc[0:NPART, j:j + 1],
                        in1=in1, op0=Alu.add, op1=Alu.subtract)
            # late groups dispatch on sync (idle once slab copies are done)
            eng = nc.scalar if g0 < 20 else nc.sync
            eng.dma_start(
                out=out_ap[g0:g0 + len(grp)].rearrange(
                    "j (k t) b w -> k j (t b w)", t=2),
                in_=og[:].rearrange("k j t b w -> k j (t b w)"))

    ctx.close()


_COMPILED = {}


def _get_compiled(schedules):
    key = hash(tuple(s for _, s in schedules))
    if key not in _COMPILED:
        nc = bacc.Bacc("TRN2", target_bir_lowering=False, debug=False,
                       num_devices=N_CORES)
        build_device_program(nc, [s for _, s in schedules])
        nc.compile()
        _COMPILED[key] = nc
    return _COMPILED[key]


def _ensure_ntff_hook():
    """The agent image's antenv lacks axon_hooks; shim it so trace=True can
    drive NTFF profiling via the boot module's ctypes hook (test-only path)."""
    import types

    try:
        from antenv.axon_hooks import get_axon_ntff_profile_hook  # noqa: F401
        return
    except ImportError:
        pass
    import antenv

    mod = types.ModuleType("antenv.axon_hooks")
    _hook = [None]
    mod.set_axon_ntff_profile_hook = lambda h: _hook.__setitem__(0, h)
    mod.get_axon_ntff_profile_hook = lambda: _hook[0]
    sys.modules["antenv.axon_hooks"] = mod
    antenv.axon_hooks = mod
    from trn_agent_boot.trn_boot import _ntff_profile_via_ctypes

    mod.set_axon_ntff_profile_hook(
        _ntff_profile_via_ctypes("/opt/axon/libaxon_pjrt.so"))


def _make_in_maps(x, thresholds, schedules):
    sdt = _band_matrices()
    ident = np.eye(NPART, dtype=np.float32)
    eye = np.stack([ident, -ident], axis=1).astype(ml_dtypes.bfloat16)
    in_maps = []
    for order, _ in schedules:
        in_maps.append({
            "x": x,
            "sdt": sdt,
            "eye": eye,
            "thr": (-thresholds[order]).reshape(1, P_CORE).astype(
                ml_dtypes.bfloat16),
        })
    return in_maps


def run(inputs: dict, trace: bool = False):
    """Run on the 8 cores. Returns (full output [B,256,H,W] f32, ns|None)."""
    x = np.asarray(inputs["x"], dtype=np.float32).reshape(B, H, W).astype(
        ml_dtypes.bfloat16)
    thresholds = np.asarray(inputs["thresholds"], np.float32)
    schedules = _host_schedule(
        inputs["offset_y1"], inputs["offset_x1"],
        inputs["offset_y2"], inputs["offset_x2"], inputs["radii"])

    nc = _get_compiled(schedules)
    in_maps = _make_in_maps(x, thresholds, schedules)

    if trace:
        _ensure_ntff_hook()
    res = run_bass_kernel_spmd(nc, in_maps, list(range(N_CORES)), trace=trace)
    full = np.empty((B, P_TOTAL, H, W), np.float32)
    for c, (order, _) in enumerate(schedules):
        oc = np.asarray(res.results[c]["out"]).astype(np.float32)
        full[:, order] = oc.transpose(2, 0, 1, 3)
    return full, res.exec_time_ns


def kernel(x, offset_x1, offset_x2, offset_y1, offset_y2, radii, thresholds,
           max_radius):
    out, _ = run({
        "x": x, "offset_x1": offset_x1, "offset_x2": offset_x2,
        "offset_y1": offset_y1, "offset_y2": offset_y2,
        "radii": radii, "thresholds": thresholds, "max_radius": max_radius,
    })
    return out


if __name__ == "__main__":
    rng = np.random.default_rng(0)
    out = kernel(
        x=rng.standard_normal((B, 1, H, W), dtype=np.float32),
        offset_x1=rng.uniform(-16, 16, P_TOTAL).astype(np.float32),
        offset_x2=rng.uniform(-16, 16, P_TOTAL).astype(np.float32),
        offset_y1=rng.uniform(-16, 16, P_TOTAL).astype(np.float32),
        offset_y2=rng.uniform(-16, 16, P_TOTAL).astype(np.float32),
        radii=rng.integers(1, 4, P_TOTAL).astype(np.int32),
        thresholds=(rng.standard_normal(P_TOTAL) * 0.1).astype(np.float32),
        max_radius=3,
    )
    print("out", out.shape, out.dtype, float(np.abs(out).max()))


# revision 4
# speedup vs baseline: 1.0086x; 1.0086x over previous
"""BAD-descriptor kernel for Trainium2 (8 NeuronCores, SPMD over pairs).

The v1 baseline (147-158us) was DMA packet-rate bound (~35.7K packets @
~60ns/pkt/engine): every pair re-gathered two 224x224 windows from a DRAM
box-mean scratch in 896B packets.  This version (~64us) keeps the box-mean
images in SBUF (bf16) in an interleaved-plane layout

  bmp[k, d-1, plane, b, c],  plane E[k]=BMP row 2k, O[k]=2k+1, Es[k]=2k+2

so a window with row-shift sy = 2q+c is partitions q..q+111, planes c..c+1.
Compute-engine APs must start at partition 0/32/64/96 (TRN2 quadrant rule),
so for q>0 the shift is materialized by a partition-shift SBUF->SBUF DMA
"slab" (112 packets x 3KB), deduplicated per (d, q) cell across the core's
64 window terms.  Pairs are ASSIGNED to cores (greedy + swap rebalance) to
minimize the worst core's slab time, since cells shared within a core are
copied once.  q==0 windows read bmp directly; column shift and plane parity
are free-dim offsets.  Per pair either

  DVE:    out = (W1 + (-thr)) - W2      (one scalar_tensor_tensor), or
  PE+ACT: psum = I*W1 + (-I)*W2; ACT drains with bias=-thr (Identity)

with ~7 pairs offloaded to the otherwise-idle PE/ACT to shorten the DVE
stream.  All shifts/radii are computed on the HOST (the offsets are kernel
inputs) and each core's 32-pair schedule is baked into an 8-way tc.Switch
on the partition id -> one SPMD program, no per-pair gathers/values_loads.
Outputs are written as bf16 (halves the flush) and upcast on the host;
end-to-end rel L2 error ~4e-3 vs the 2e-2 gate.

Box-mean build (stage B), pipelined per radius so d=1 pairs start earliest:
bf16 horizontal 7-tap chains on DVE (edge pads on GpSimd), vertical taps +
row-replicate clipping + 1/area baked into bf16 band matrices on PE
(2 K-tiles x 9 (plane,d) matmuls), psum drained to bmp by ACT.  Slab DMAs
dispatch on SP, early output DMAs on ACT, late ones on SP.
"""

import sys

sys.path.insert(0, "/opt/trn_rl_repo")

from contextlib import ExitStack

import numpy as np
import ml_dtypes

import concourse.bass as bass
import concourse.bacc as bacc
import concourse.mybir as mybir
import concourse.tile as tile
from concourse.bass_utils import run_bass_kernel_spmd

B = 2
H = W = 224
P_TOTAL = 256
N_CORES = 8
P_CORE = P_TOTAL // N_CORES  # 32
PAD = 16
RMAX = 3
HP = H + 2 * PAD  # 256 padded rows/cols
XPAD = RMAX  # 3: replicate pad for the +-d box samples
XW = W + 2 * XPAD  # 230
F32 = mybir.dt.float32
BF16 = mybir.dt.bfloat16
NPART = 112
SLAB_SLOTS = 24  # LRU capacity for multi-use (d,q) slabs: 24 x 3KB/partition


def _host_schedule(offset_y1, offset_x1, offset_y2, offset_x2, radii):
    """Per-core pair schedules (d, sy1, sx1, sy2, sx2).

    Pairs are ASSIGNED to cores greedily so that pairs sharing a shifted
    window cell (d, sy) land on the same core — each distinct cell costs one
    229KB slab copy, the dominant stage-C DMA traffic.  q==0 cells are free
    (direct bmp reads).
    """

    def prep(off):
        fo = np.floor(np.asarray(off, np.float32).astype(np.float64))
        return (np.clip(fo, -PAD, PAD) + PAD).astype(np.int64)  # [0,32]

    sy1, sx1 = prep(offset_y1), prep(offset_x1)
    sy2, sx2 = prep(offset_y2), prep(offset_x2)
    d = np.clip(np.asarray(radii).astype(np.int64), 1, RMAX)

    def cells(p):
        s = set()
        for sy in (int(sy1[p]), int(sy2[p])):
            if (sy >> 1) != 0:
                s.add((int(d[p]), sy >> 1))
        return s

    from collections import Counter
    freq = Counter()
    for p in range(P_TOTAL):
        for cell in cells(p):
            freq[cell] += 1

    def slab_bytes(take):
        """Time proxy (ns-ish): dispatch serialization + transfer bytes."""
        cnt = Counter()
        for p in take:
            for cell in cells(p):
                cnt[cell] += 1
        b = sum(344064 if v > 1 else 200704 for v in cnt.values())
        return len(cnt) * 900 + b // 300

    # sequential greedy builds tight clusters (later cores get scattered
    # leftovers), then pairwise swaps rebalance the worst core (exec time
    # is the MAX over cores)
    remaining = set(range(P_TOTAL))
    takes = []
    for c in range(N_CORES):
        take, open_cells = [], set()
        while len(take) < P_CORE:
            best, best_key = None, None
            for p in remaining:
                cs = cells(p)
                reuse = sum(1 for x in cs if x in open_cells)
                new = [x for x in cs if x not in open_cells]
                key = (reuse, -len(new), sum(freq[x] for x in new))
                if best_key is None or key > best_key:
                    best, best_key = p, key
            take.append(best)
            remaining.discard(best)
            for x in cells(best):
                open_cells.add(x)
                freq[x] -= 1
        takes.append(take)

    cur = [slab_bytes(t) for t in takes]
    for _ in range(300):
        hi = int(np.argmax(cur))
        best = None
        for lo in range(N_CORES):
            if lo == hi:
                continue
            for i in range(P_CORE):
                for jj in range(P_CORE):
                    t1 = takes[hi][:i] + takes[hi][i + 1:] + [takes[lo][jj]]
                    t2 = takes[lo][:jj] + takes[lo][jj + 1:] + [takes[hi][i]]
                    m = max(slab_bytes(t1), slab_bytes(t2))
                    if m < max(cur[hi], cur[lo]) and (
                            best is None or m < best[0]):
                        best = (m, lo, i, jj)
        if best is None:
            break
        _, lo, i, jj = best
        takes[hi][i], takes[lo][jj] = takes[lo][jj], takes[hi][i]
        cur[hi] = slab_bytes(takes[hi])
        cur[lo] = slab_bytes(takes[lo])

    cores = []
    for c in range(N_CORES):
        # within a core, order pairs by radius so stage C can start as soon
        # as the d=1 planes are built, preserving cluster adjacency second
        order = np.array(sorted(takes[c], key=lambda p: int(d[p])))
        sched = tuple(
            (int(d[p]), int(sy1[p]), int(sx1[p]), int(sy2[p]), int(sx2[p]))
            for p in order
        )
        cores.append((order, sched))
    return cores


def _band_matrices() -> np.ndarray:
    """sdt[kt, kr, 3*pi + d-1, m]: vertical (2d+1)-tap band matrices with the
    row-replicate clipping and the 1/(2d+1)^2 box area baked in.

    Plane pi in {0:E, 1:O, 2:Es} holds BMP row r = 2m + pi at partition m;
    BMP row r represents BM row h = clip(r-16, 0, 223) whose box mean is
    (1/area) * sum_i hs_d[clip(h+i, 0, 223)].  K-tile 0 = x rows 0..127,
    K-tile 1 = x rows 96..223 (coefficients split at row 128).
    """
    sdt = np.zeros((2, 128, 9, 128), np.float32)
    for dd in (1, 2, 3):
        inv_area = 1.0 / float((2 * dd + 1) ** 2)
        for pi in range(3):
            col = 3 * pi + (dd - 1)
            for m in range(128):
                r = 2 * m + pi
                h = min(max(r - PAD, 0), H - 1)
                for i in range(-dd, dd + 1):
                    xr = min(max(h + i, 0), H - 1)
                    if xr < 128:
                        sdt[0, xr, col, m] += inv_area
                    else:
                        sdt[1, xr - 96, col, m] += inv_area
    return sdt.astype(ml_dtypes.bfloat16)


def build_device_program(nc: bacc.Bacc, schedules):
    x_ap = nc.dram_tensor("x", [B, H, W], BF16, kind="ExternalInput").ap()
    sdt_ap = nc.dram_tensor("sdt", [2, 128, 9, 128], BF16,
                            kind="ExternalInput").ap()
    eye_ap = nc.dram_tensor("eye", [NPART, 2, NPART], BF16,
                            kind="ExternalInput").ap()
    thr_ap = nc.dram_tensor("thr", [1, P_CORE], BF16,
                            kind="ExternalInput").ap()
    out_ap = nc.dram_tensor("out", [P_CORE, H, B, W], BF16,
                            kind="ExternalOutput").ap()
    with tile.TileContext(nc) as tc:
        build_kernel(tc, out_ap, x_ap, sdt_ap, eye_ap, thr_ap, schedules)
    return nc


def build_kernel(tc, out_ap, x_ap, sdt_ap, eye_ap, thr_ap, schedules):
    nc = tc.nc
    EngT = mybir.EngineType
    Alu = mybir.AluOpType
    Act = mybir.ActivationFunctionType

    ctx = ExitStack()
    const_pool = ctx.enter_context(tc.tile_pool(name="const", bufs=1))
    work_pool = ctx.enter_context(tc.tile_pool(name="work", bufs=1))
    psum_pool = ctx.enter_context(tc.tile_pool(name="psum", bufs=2,
                                               space="PSUM"))
    slab_pool = ctx.enter_context(tc.tile_pool(name="slab", bufs=1))
    o_pool = ctx.enter_context(tc.tile_pool(name="outt", bufs=6))

    # ---------------- inputs / constants ----------------
    thr_bc = const_pool.tile([NPART, P_CORE], BF16, tag="thr_bc")
    nc.scalar.dma_start(out=thr_bc[:],
                        in_=thr_ap[0:1, :].to_broadcast((NPART, P_CORE)))

    part_rows = ((0, 128), (96, 128))  # x K-tiles (overlapping rows 96..127)
    xt = work_pool.tile([128, 2, B, XW], BF16, tag="xt")
    for j, (r0, nr) in enumerate(part_rows):
        eng = nc.sync if j == 0 else nc.scalar
        eng.dma_start(out=xt[:, j, :, XPAD:XPAD + W],
                      in_=x_ap[:, r0:r0 + nr, :].rearrange("b r w -> r b w"))
    nc.vector.tensor_copy(
        out=xt[:, :, :, 0:XPAD],
        in_=xt[:, :, :, XPAD:XPAD + 1].to_broadcast((128, 2, B, XPAD)))
    nc.vector.tensor_copy(
        out=xt[:, :, :, XPAD + W:],
        in_=xt[:, :, :, XPAD + W - 1:XPAD + W].to_broadcast((128, 2, B, XPAD)))

    sdt_sb = const_pool.tile([128, 2, 9, 128], BF16, tag="sdt")
    nc.sync.dma_start(out=sdt_sb[:, 0], in_=sdt_ap[0])
    nc.scalar.dma_start(out=sdt_sb[:, 1], in_=sdt_ap[1])

    # ------- box-mean build, pipelined per radius so d=1 lands first ------
    # hs is computed at VALID centers c in [0,224) (samples clip via the
    # replicate-padded xt); the 16-wide window margins replicate the EDGE
    # hs value (reference clips the box center, then samples around it).
    # bmp[k, d-1, plane, b, c]: plane-adjacent-within-d so a (d, parity) slab
    # source bmp[q:q+112, d-1, c:c+2, :, :] is 2KB contiguous per partition.
    bmp = const_pool.tile([128, 3, 3, B, HP], BF16, tag="bmp")
    hprev = None
    CHAIN = {1: (2, 4), 2: (1, 5), 3: (0, 6)}  # xt slice offsets per radius
    for dd in (1, 2, 3):
        sl = lambda o: xt[:, :, :, o:o + W]
        oa, ob = CHAIN[dd]
        hf = work_pool.tile([128, 2, B, HP], BF16, tag=f"h{dd}")
        tt = work_pool.tile([128, 2, B, W], BF16, tag=f"t{dd}")
        hv = lambda h: h[:, :, :, PAD:PAD + W]
        nc.vector.tensor_tensor(out=tt[:], in0=sl(oa), in1=sl(ob), op=Alu.add)
        if dd == 1:
            nc.vector.tensor_tensor(out=hv(hf), in0=tt[:], in1=sl(3),
                                    op=Alu.add)
        else:
            nc.vector.tensor_tensor(out=hv(hf), in0=hv(hprev), in1=tt[:],
                                    op=Alu.add)
        hprev = hf
        nc.gpsimd.tensor_copy(
            out=hf[:, :, :, 0:PAD],
            in_=hf[:, :, :, PAD:PAD + 1].to_broadcast((128, 2, B, PAD)))
        nc.gpsimd.tensor_copy(
            out=hf[:, :, :, PAD + W:],
            in_=hf[:, :, :, PAD + W - 1:PAD + W].to_broadcast((128, 2, B, PAD)))
        for pi in range(3):
            col = 3 * pi + (dd - 1)
            ps = psum_pool.tile([128, B * HP], F32, tag=f"ps{pi % 2}")
            nc.tensor.matmul(out=ps[:], lhsT=sdt_sb[:, 0, col, :],
                             rhs=hf[:, 0, :, :].rearrange("r b c -> r (b c)"),
                             start=True, stop=False)
            nc.tensor.matmul(out=ps[:], lhsT=sdt_sb[:, 1, col, :],
                             rhs=hf[:, 1, :, :].rearrange("r b c -> r (b c)"),
                             start=False, stop=True)
            nc.scalar.activation(bmp[:, dd - 1, pi, :, :],
                                 ps[:].rearrange("m (b c) -> m b c", b=B),
                                 Act.Copy, scale=1.0)
    # eye is only needed by the PE-offload pairs (~35us in): load late so it
    # doesn't occupy the sync stream ahead of the first slab dispatches
    eye_sb = const_pool.tile([NPART, 2, NPART], BF16, tag="eye")
    nc.scalar.dma_start(out=eye_sb[:], in_=eye_ap[:])

    # ---------------- stage C: per-core static schedules via Switch -------
    pid = nc.partition_id(engines=[EngT.DVE, EngT.SP, EngT.Activation, EngT.PE])
    GROUP = 4  # pairs per merged output DMA
    for case in tc.Switch(pid, n=N_CORES):
        sched = schedules[case]
        # slab usage counts: single-use slabs are parity/col-sliced,
        # multi-use slabs hold all 3 planes of one radius at one q (3KB/
        # partition, one dispatch serves both parities and any sx)
        from collections import Counter
        cnt = Counter()
        for (dd, sy1, sx1, sy2, sx2) in sched:
            for sy in (sy1, sy2):
                if (sy >> 1) != 0:
                    cnt[(dd, sy >> 1)] += 1
        cache = {}
        lru = []  # keys, most recent last
        free_slots = list(range(SLAB_SLOTS))

        def window(dd, sy, sx):
            q, c = sy >> 1, sy & 1
            if q == 0:
                return bmp[0:NPART, dd - 1, c:c + 2, :, sx:sx + W]
            key = (dd, q)
            if cnt[key] == 1:
                st = slab_pool.tile([NPART, 2, B, W], BF16, tag="ss",
                                    bufs=6, name="sslab")
                nc.sync.dma_start(
                    out=st[:],
                    in_=bmp[q:q + NPART, dd - 1, c:c + 2, :, sx:sx + W])
                return st[:]
            if key in cache:
                lru.remove(key)
                lru.append(key)
                slab = cache[key]
            else:
                if free_slots:
                    slot = free_slots.pop()
                else:
                    old = lru.pop(0)
                    slot = cache.pop(old)[0]
                st = slab_pool.tile([NPART, 3, B, HP], BF16,
                                    tag=f"sl{slot}", bufs=1, name=f"slab{slot}")
                nc.sync.dma_start(out=st[:],
                                  in_=bmp[q:q + NPART, dd - 1, :, :, :])
                cache[key] = (slot, st)
                lru.append(key)
                slab = cache[key]
            return slab[1][:, c:c + 2, :, sx:sx + W]

        # groups of 4 early; smaller at the tail so the final out-DMA (which
        # serializes after the last STT) is short
        bounds = list(range(0, 28, 2)) + [28, 29, 30, 31]
        for bi, g0 in enumerate(bounds):
            g1 = bounds[bi + 1] if bi + 1 < len(bounds) else len(sched)
            grp = sched[g0:g1]
            og = o_pool.tile([NPART, len(grp), 2, B, W], BF16, tag="o")
            for gi, (dd, sy1, sx1, sy2, sx2) in enumerate(grp):
                j = g0 + gi
                in0 = window(dd, sy1, sx1)
                in1 = window(dd, sy2, sx2)
                if j % 5 == 2 or j in (29, 31):
                    # offload to PE: psum = I*W1 + (-I)*W2 (one bank per
                    # plane-half), ACT drains with bias = -thr + bf16 cast
                    for t in range(2):
                        pp = psum_pool.tile([NPART, B, W], F32,
                                            tag=f"pp{t}", bufs=2)
                        nc.tensor.matmul(out=pp[:], lhsT=eye_sb[:, 0, :],
                                         rhs=in0[:, t], start=True, stop=False)
                        nc.tensor.matmul(out=pp[:], lhsT=eye_sb[:, 1, :],
                                         rhs=in1[:, t], start=False, stop=True)
                        nc.scalar.activation(og[:, gi, t], pp[:], Act.Identity,
                                             bias=thr_bc[0:NPART, j:j + 1],
                                             scale=1.0)
                else:
                    nc.vector.scalar_tensor_tensor(
                        out=og[:, gi], in0=in0,
                        scalar=thr_bc[0:NPART, j:j + 1],
                        in1=in1, op0=Alu.add, op1=Alu.subtract)
            # late groups dispatch on sync (idle once slab copies are done)
            eng = nc.scalar if g0 < 20 else nc.sync
            eng.dma_start(
                out=out_ap[g0:g0 + len(grp)].rearrange(
                    "j (k t) b w -> k j (t b w)", t=2),
                in_=og[:].rearrange("k j t b w -> k j (t b w)"))

    ctx.close()


_COMPILED = {}


def _get_compiled(schedules):
    key = hash(tuple(s for _, s in schedules))
    if key not in _COMPILED:
        nc = bacc.Bacc("TRN2", target_bir_lowering=False, debug=False,
                       num_devices=N_CORES)
        build_device_program(nc, [s for _, s in schedules])
        nc.compile()
        _COMPILED[key] = nc
    return _COMPILED[key]


def _ensure_ntff_hook():
    """The agent image's antenv lacks axon_hooks; shim it so trace=True can
    drive NTFF profiling via the boot module's ctypes hook (test-only path)."""
    import types

    try:
        from antenv.axon_hooks import get_axon_ntff_profile_hook  # noqa: F401
        return
    except ImportError:
        pass
    import antenv

    mod = types.ModuleType("antenv.axon_hooks")
    _hook = [None]
    mod.set_axon_ntff_profile_hook = lambda h: _hook.__setitem__(0, h)
    mod.get_axon_ntff_profile_hook = lambda: _hook[0]
    sys.modules["antenv.axon_hooks"] = mod
    antenv.axon_hooks = mod
    from trn_agent_boot.trn_boot import _ntff_profile_via_ctypes

    mod.set_axon_ntff_profile_hook(
        _ntff_profile_via_ctypes("/opt/axon/libaxon_pjrt.so"))


def _make_in_maps(x, thresholds, schedules):
    sdt = _band_matrices()
    ident = np.eye(NPART, dtype=np.float32)
    eye = np.stack([ident, -ident], axis=1).astype(ml_dtypes.bfloat16)
    in_maps = []
    for order, _ in schedules:
        in_maps.append({
            "x": x,
            "sdt": sdt,
            "eye": eye,
            "thr": (-thresholds[order]).reshape(1, P_CORE).astype(
                ml_dtypes.bfloat16),
        })
    return in_maps


def run(inputs: dict, trace: bool = False):
    """Run on the 8 cores. Returns (full output [B,256,H,W] f32, ns|None)."""
    x = np.asarray(inputs["x"], dtype=np.float32).reshape(B, H, W).astype(
        ml_dtypes.bfloat16)
    thresholds = np.asarray(inputs["thresholds"], np.float32)
    schedules = _host_schedule(
        inputs["offset_y1"], inputs["offset_x1"],
        inputs["offset_y2"], inputs["offset_x2"], inputs["radii"])

    nc = _get_compiled(schedules)
    in_maps = _make_in_maps(x, thresholds, schedules)

    if trace:
        _ensure_ntff_hook()
    res = run_bass_kernel_spmd(nc, in_maps, list(range(N_CORES)), trace=trace)
    full = np.empty((B, P_TOTAL, H, W), np.float32)
    for c, (order, _) in enumerate(schedules):
        oc = np.asarray(res.results[c]["out"]).astype(np.float32)
        full[:, order] = oc.transpose(2, 0, 1, 3)
    return full, res.exec_time_ns


def kernel(x, offset_x1, offset_x2, offset_y1, offset_y2, radii, thresholds,
           max_radius):
    out, _ = run({
        "x": x, "offset_x1": offset_x1, "offset_x2": offset_x2,
        "offset_y1": offset_y1, "offset_y2": offset_y2,
        "radii": radii, "thresholds": thresholds, "max_radius": max_radius,
    })
    return out


if __name__ == "__main__":
    rng = np.random.default_rng(0)
    out = kernel(
        x=rng.standard_normal((B, 1, H, W), dtype=np.float32),
        offset_x1=rng.uniform(-16, 16, P_TOTAL).astype(np.float32),
        offset_x2=rng.uniform(-16, 16, P_TOTAL).astype(np.float32),
        offset_y1=rng.uniform(-16, 16, P_TOTAL).astype(np.float32),
        offset_y2=rng.uniform(-16, 16, P_TOTAL).astype(np.float32),
        radii=rng.integers(1, 4, P_TOTAL).astype(np.int32),
        thresholds=(rng.standard_normal(P_TOTAL) * 0.1).astype(np.float32),
        max_radius=3,
    )
    print("out", out.shape, out.dtype, float(np.abs(out).max()))
